# revision 5
# baseline (speedup 1.0000x reference)
"""Trainium2 Bass kernel for nn_CoreDecoderStatefull (single-step stateful decoder).

Structure: dense -> 5x [GRU cell -> GLU -> concat -> stateful conv1d(k=2) -> concat]
-> output projection.  batch=1, seq=1: every matmul is a vector-matrix product.

Strategy (sharding hint: not shardable -> replicate on all 8 cores, read core 0):
  * All vectors live in SBUF as columns [<=128 partitions, 1].
  * Every matmul is W.T-stationary: lhsT = W.T block [K<=128, M<=96], moving
    rhs = x column [K, 1]; output is a PSUM column -> no transposes anywhere.
  * All weights are pre-transposed / permuted / packed host-side into a few
    [128, C] DRAM slabs that DMA straight into SBUF (memory-bound roofline).
  * The growing concat vector x lives as 6 chunk-columns of a [128, 6] tile:
      chunk c rows 0:96  = x0 (c=0) or glu output g_c
      chunk c rows 96:128 = conv output cv_{c+1}
    Weight rows are permuted host-side to match this storage order.
  * Biases: folded into augmented constant-1 rows (dense: z row 80; GRU r/z/n
    h-terms: h row 96; out: X chunk5 row 96) or into ACT bias APs (gi n-gate
    bias, conv bias+c-state term).
  * Noise sites are deterministic (jax fold_in(key(42), i)) -> precomputed
    host-side and DMA'd as columns.
  * Input-only matmuls (dense, GRU h-terms, conv c-state taps) run early under
    the weight-DMA shadow and are copied to SBUF so PSUM needs only 8 banks.
"""

import os
import numpy as np
from contextlib import ExitStack

GD = [96, 224, 352, 480, 608]   # GRU input dims per stage
CD = [192, 320, 448, 576, 704]  # conv input dims per stage
N_CORES = 8


# ---------------------------------------------------------------------------
# reference x-vector index mapping
# ---------------------------------------------------------------------------
# Reference concat order: x0(96), g1(96), cv1(32), g2, cv2, ..., g5, cv5 (736).
# Our storage: chunk c rows 0:96 = (x0 if c==0 else g_c); rows 96:128 = cv_{c+1}.
def _refidx(c: int, r: int) -> int:
    if r < 96:
        return r if c == 0 else GD[c - 1] + r      # x0 or g_c
    assert c <= 4
    return CD[c] + (r - 96)                        # cv_{c+1}


def _gi_chunks(s):   # GRU input-side chunks for stage s (1-based)
    return [(c, 128) for c in range(s - 1)] + [(s - 1, 96)]


def _cvx_chunks(s):  # conv current-input (tap 1) chunks for stage s
    return [(c, 128) for c in range(s - 1)] + [(s - 1, 96), (s, 96)]


_OUT_CHUNKS = [(c, 128) for c in range(5)] + [(5, 97)]  # row 96 = 1.0 (b_out)


# ---------------------------------------------------------------------------
# static layout: which (slab, col, rows, ncols) each weight block occupies
# ---------------------------------------------------------------------------
def _layout():
    wt = {}            # name -> (slab, col, rows, ncols)
    slab_cols = [0] * 7

    def put(name, slab, rows, ncols):
        wt[name] = (slab, slab_cols[slab], rows, ncols)
        slab_cols[slab] += ncols

    # slab 0: everything needed early (input-only matmuls), in consumption order
    put("dense", 0, 81, 96)
    for s in range(1, 6):
        for j in range(3):
            put(f"gh{s}_{j}", 0, 97, 96)
        cd = CD[s - 1]
        for j in range((cd + 127) // 128):
            put(f"cvc{s}_{j}", 0, min(128, cd - 128 * j), 32)
    # slabs 1..5: stage s chain weights
    for s in range(1, 6):
        for (c, rows) in _gi_chunks(s):
            for j in range(3):
                put(f"gi{s}_{c}_{j}", s, rows, 96)
        put(f"glu{s}", s, 96, 96)
        for (c, rows) in _cvx_chunks(s):
            put(f"cvx{s}_{c}", s, rows, 32)
    # slab 6: output projection
    for (c, rows) in _OUT_CHUNKS:
        put(f"out{c}", 6, rows, 80)

    st = {}            # name -> (col, rows)
    scol = 0

    def sput(name, rows):
        nonlocal scol
        st[name] = (scol, rows)
        scol += 1

    sput("z", 81)                      # z (80) + 1.0
    for s in range(1, 6):
        sput(f"h{s}", 97)              # h (96) + 1.0
    for s in range(1, 6):
        cd = CD[s - 1]
        for j in range((cd + 127) // 128):
            sput(f"c{s}_{j}", min(128, cd - 128 * j))
    sput("n0", 96)
    for s in range(1, 6):
        sput(f"nh{s}", 96)
        sput(f"ng{s}", 96)
        sput(f"ncv{s}", 32)
    for s in range(1, 6):
        sput(f"bin{s}", 96)            # bi[192:288] (gi n-gate bias)
    for s in range(1, 6):
        sput(f"cb{s}", 32)             # conv bias
    return wt, st, slab_cols, scol


_WT, _ST, _SLAB_COLS, _ST_COLS = _layout()


# ---------------------------------------------------------------------------
# host-side packing
# ---------------------------------------------------------------------------
def _noise_vectors():
    import jax
    import jax.numpy as jnp

    vs = {}
    for i in range(16):
        n = 96 if (i == 0 or i % 3 != 0) else 32
        u = jax.random.uniform(
            jax.random.fold_in(jax.random.key(42), i), (1, n), dtype=jnp.float32
        )
        vs[i] = (np.asarray(u).reshape(-1) - 0.5) / np.float32(127.0)
    return vs


def _pack(inp):
    slabs = [np.zeros((128, c), np.float32) for c in _SLAB_COLS]
    stile = np.zeros((128, _ST_COLS), np.float32)

    def wfill(name, block):
        slab, col, rows, ncols = _WT[name]
        assert block.shape == (rows, ncols), (name, block.shape, (rows, ncols))
        slabs[slab][:rows, col:col + ncols] = block

    def sfill(name, vec):
        col, rows = _ST[name]
        assert vec.shape == (rows,), (name, vec.shape, rows)
        stile[:rows, col] = vec

    f32 = np.float32
    # dense: [81, 96] = [w_dense.T ; b_dense]
    blk = np.zeros((81, 96), f32)
    blk[:80] = inp["w_dense"].T
    blk[80] = inp["b_dense"]
    wfill("dense", blk)

    for s in range(1, 6):
        wh, bi, bh = inp[f"g{s}_wh"], inp[f"g{s}_bi"], inp[f"g{s}_bh"]
        for j in range(3):
            blk = np.zeros((97, 96), f32)
            blk[:96] = wh[96 * j:96 * (j + 1), :].T
            # gates r,z: fold bi+bh into the h-aug row; gate n: bh only
            blk[96] = (bi + bh)[96 * j:96 * (j + 1)] if j < 2 else bh[192:288]
            wfill(f"gh{s}_{j}", blk)
        cw0 = inp[f"cv{s}_w"][:, :, 0]
        cd = CD[s - 1]
        for j in range((cd + 127) // 128):
            rows = min(128, cd - 128 * j)
            wfill(f"cvc{s}_{j}", cw0[:, 128 * j:128 * j + rows].T.astype(f32))

    for s in range(1, 6):
        wi = inp[f"g{s}_wi"]
        for (c, rows) in _gi_chunks(s):
            ridx = [_refidx(c, r) for r in range(rows)]
            for j in range(3):
                wfill(f"gi{s}_{c}_{j}", wi[96 * j:96 * (j + 1), ridx].T.astype(f32))
        wfill(f"glu{s}", inp[f"glu{s}_w"].T.astype(f32))
        cw1 = inp[f"cv{s}_w"][:, :, 1]
        for (c, rows) in _cvx_chunks(s):
            ridx = [_refidx(c, r) for r in range(rows)]
            wfill(f"cvx{s}_{c}", cw1[:, ridx].T.astype(f32))

    w_out, b_out = inp["w_out"], inp["b_out"]
    for (c, rows) in _OUT_CHUNKS:
        if c < 5:
            ridx = [_refidx(c, r) for r in range(rows)]
            wfill(f"out{c}", w_out[:, ridx].T.astype(f32))
        else:
            blk = np.zeros((97, 80), f32)
            blk[:96] = w_out[:, 608:704].T
            blk[96] = b_out
            wfill(f"out{c}", blk)

    # state tile
    zv = np.zeros(81, f32)
    zv[:80] = inp["z"].reshape(-1)
    zv[80] = 1.0
    sfill("z", zv)
    for s in range(1, 6):
        hv = np.zeros(97, f32)
        hv[:96] = inp[f"h{s}"].reshape(-1)
        hv[96] = 1.0
        sfill(f"h{s}", hv)
        cv = inp[f"c{s}"].reshape(-1)
        cd = CD[s - 1]
        for j in range((cd + 127) // 128):
            rows = min(128, cd - 128 * j)
            sfill(f"c{s}_{j}", cv[128 * j:128 * j + rows].astype(f32))
    nv = _noise_vectors()
    sfill("n0", nv[0])
    for s in range(1, 6):
        sfill(f"nh{s}", nv[3 * s - 2])
        sfill(f"ng{s}", nv[3 * s - 1])
        sfill(f"ncv{s}", nv[3 * s])
    for s in range(1, 6):
        sfill(f"bin{s}", inp[f"g{s}_bi"][192:288].astype(f32))
        sfill(f"cb{s}", inp[f"cv{s}_b"].astype(f32))

    m = {f"wslab{i}": slabs[i] for i in range(7)}
    m["stile"] = stile
    return m


# ---------------------------------------------------------------------------
# device program
# ---------------------------------------------------------------------------
def _build_nc(loop_iters=None):
    from concourse import bacc, tile, mybir
    import concourse.bass as bass

    F32 = mybir.dt.float32
    AF = mybir.ActivationFunctionType
    OP = mybir.AluOpType

    nc = bacc.Bacc("TRN2", target_bir_lowering=False, debug=False,
                   num_devices=N_CORES)
    wdram = [nc.dram_tensor(f"wslab{i}", [128, _SLAB_COLS[i]], F32,
                            kind="ExternalInput") for i in range(7)]
    sdram = nc.dram_tensor("stile", [128, _ST_COLS], F32, kind="ExternalInput")
    ydram = nc.dram_tensor("y", [80, 1], F32, kind="ExternalOutput")

    with tile.TileContext(nc) as tc, ExitStack() as ctx:
        wpool = ctx.enter_context(tc.tile_pool(name="wpool", bufs=1))
        spool = ctx.enter_context(tc.tile_pool(name="spool", bufs=1))
        work = ctx.enter_context(tc.tile_pool(name="work", bufs=1))
        pearly = ctx.enter_context(tc.tile_pool(name="pearly", bufs=2, space="PSUM"))
        pgi = ctx.enter_context(tc.tile_pool(name="pgi", bufs=2, space="PSUM"))
        pglu = ctx.enter_context(tc.tile_pool(name="pglu", bufs=2, space="PSUM"))
        pconv = ctx.enter_context(tc.tile_pool(name="pconv", bufs=2, space="PSUM"))

        if loop_iters is not None:
            ctx.enter_context(tc.For_i(0, loop_iters, 1))

        # --- ACT table prefetch: touch sigmoid+tanh before any real work ---
        warm = work.tile([1, 1], F32, tag="warm", name="warm")
        nc.vector.memset(warm[:], 0.0)
        warm2 = work.tile([1, 1], F32, tag="warm2", name="warm2")
        nc.scalar.activation(warm2[:], warm[:], AF.Sigmoid)
        nc.scalar.activation(warm2[:], warm2[:], AF.Tanh)

        # --- tiles ---
        X = work.tile([128, 6], F32, tag="X", name="X")
        nc.vector.memset(X[96:97, 5:6], 1.0)
        stile = spool.tile([128, _ST_COLS], F32, tag="stile", name="stile")
        nc.sync.dma_start(out=stile[:], in_=sdram[:])
        wt = []
        for i in range(7):
            t = wpool.tile([128, _SLAB_COLS[i]], F32, tag=f"w{i}", name=f"wt{i}")
            nc.sync.dma_start(out=t[:], in_=wdram[i][:])
            wt.append(t)

        def W(name):
            slab, col, rows, ncols = _WT[name]
            return wt[slab][0:rows, col:col + ncols]

        def S(name, rows=None):
            col, r = _ST[name]
            if rows is not None:
                r = rows
            return stile[0:r, col:col + 1]

        # --- early input-only matmuls (overlap the big weight DMA) ---
        pd = pearly.tile([96, 3], F32, tag="early", name="pdense")
        nc.tensor.matmul(pd[:, 0:1], W("dense"), S("z"), start=True, stop=True)
        x0t = work.tile([96, 1], F32, tag="x0t", name="x0t")
        nc.scalar.activation(x0t[:], pd[:96, 0:1], AF.Tanh)
        tx0 = work.tile([96, 1], F32, tag="tx0", name="tx0")
        nc.vector.tensor_scalar(tx0[:], x0t[:], S("n0"), -1.0, OP.add, OP.max)
        nc.vector.tensor_scalar(X[0:96, 0:1], tx0[:], 1.0, None, OP.min)

        ghsb, cpart = {}, {}
        for s in range(1, 6):
            pg = pearly.tile([96, 3], F32, tag="early", name=f"pgh{s}")
            for j in range(3):
                nc.tensor.matmul(pg[:, j:j + 1], W(f"gh{s}_{j}"), S(f"h{s}"),
                                 start=True, stop=True)
            g = work.tile([96, 3], F32, tag=f"ghsb{s}", name=f"ghsb{s}")
            nc.vector.tensor_copy(g[:], pg[:])
            ghsb[s] = g
            pc = pearly.tile([96, 3], F32, tag="early", name=f"pcvc{s}")
            cd = CD[s - 1]
            nch = (cd + 127) // 128
            for j in range(nch):
                rows = min(128, cd - 128 * j)
                nc.tensor.matmul(pc[0:32, 0:1], W(f"cvc{s}_{j}"), S(f"c{s}_{j}"),
                                 start=(j == 0), stop=(j == nch - 1))
            cp = work.tile([32, 1], F32, tag=f"cpart{s}", name=f"cpart{s}")
            # conv c-state partial + conv bias, folded for the later ACT bias
            nc.vector.tensor_scalar(cp[:], pc[0:32, 0:1], S(f"cb{s}"), None, OP.add)
            cpart[s] = cp

        # --- sequential chain ---
        for s in range(1, 6):
            P = pgi.tile([96, 3], F32, tag="gi", name=f"Pgi{s}")
            chunks = _gi_chunks(s)
            for j in (0, 2, 1):  # r first (unblocks ACT), then n (t2), then z
                for idx, (c, rows) in enumerate(chunks):
                    nc.tensor.matmul(P[:, j:j + 1], W(f"gi{s}_{c}_{j}"),
                                     X[0:rows, c:c + 1],
                                     start=(idx == 0), stop=(idx == len(chunks) - 1))
            r = work.tile([96, 1], F32, tag="r", name=f"r{s}")
            z = work.tile([96, 1], F32, tag="zz", name=f"z{s}")
            # r = sigmoid(gi_r + gh_r(+biases)) ; z likewise
            nc.scalar.activation(r[:], P[:, 0:1], AF.Sigmoid, bias=ghsb[s][:, 0:1])
            nc.scalar.activation(z[:], P[:, 1:2], AF.Sigmoid, bias=ghsb[s][:, 1:2])
            t2 = work.tile([96, 1], F32, tag="t2", name=f"t2_{s}")
            # t2 = gh_n * r + gi_n
            nc.vector.scalar_tensor_tensor(t2[:], ghsb[s][:, 2:3], r[:], P[:, 2:3],
                                           OP.mult, OP.add)
            c_ = work.tile([96, 1], F32, tag="c_", name=f"c{s}_")
            nc.scalar.activation(c_[:], t2[:], AF.Tanh, bias=S(f"bin{s}"))
            cn = work.tile([96, 1], F32, tag="cn", name=f"cn{s}")
            nc.vector.tensor_add(cn[:], c_[:], S(f"nh{s}"))
            hnp = work.tile([96, 1], F32, tag="hnp", name=f"hnp{s}")
            # hnp = (h - c) * z
            nc.vector.scalar_tensor_tensor(hnp[:], S(f"h{s}", 96), c_[:], z[:],
                                           OP.subtract, OP.mult)
            t3 = work.tile([96, 1], F32, tag="t3", name=f"t3_{s}")
            nc.vector.tensor_scalar(t3[:], hnp[:], cn[:], -1.0, OP.add, OP.max)
            hn3 = work.tile([96, 1], F32, tag="hn3", name=f"hn3_{s}")
            nc.vector.tensor_scalar(hn3[:], t3[:], 1.0, None, OP.min)

            Q = pglu.tile([96, 1], F32, tag="glu", name=f"Q{s}")
            nc.tensor.matmul(Q[:], W(f"glu{s}"), hn3[:], start=True, stop=True)
            sg = work.tile([96, 1], F32, tag="sg", name=f"sg{s}")
            nc.scalar.activation(sg[:], Q[:], AF.Sigmoid)
            g0 = work.tile([96, 1], F32, tag="g0", name=f"g0_{s}")
            # g0 = sg * hn3 + noise
            nc.vector.scalar_tensor_tensor(g0[:], sg[:], hn3[:], S(f"ng{s}"),
                                           OP.mult, OP.add)
            nc.vector.tensor_scalar(X[0:96, s:s + 1], g0[:], -1.0, 1.0,
                                    OP.max, OP.min)

            R = pconv.tile([32, 1], F32, tag="cv", name=f"R{s}")
            cchunks = _cvx_chunks(s)
            for idx, (c, rows) in enumerate(cchunks):
                nc.tensor.matmul(R[:], W(f"cvx{s}_{c}"), X[0:rows, c:c + 1],
                                 start=(idx == 0), stop=(idx == len(cchunks) - 1))
            cv = work.tile([32, 1], F32, tag="cv_", name=f"cv{s}_")
            nc.scalar.activation(cv[:], R[:], AF.Tanh, bias=cpart[s][:])
            cv0 = work.tile([32, 1], F32, tag="cv0", name=f"cv0_{s}")
            nc.vector.tensor_scalar(cv0[:], cv[:], S(f"ncv{s}"), -1.0, OP.add, OP.max)
            nc.vector.tensor_scalar(X[96:128, s - 1:s], cv0[:], 1.0, None, OP.min)

        # --- output projection ---
        O = pglu.tile([80, 1], F32, tag="glu", name="Oout")
        for idx, (c, rows) in enumerate(_OUT_CHUNKS):
            nc.tensor.matmul(O[:], W(f"out{c}"), X[0:rows, c:c + 1],
                             start=(idx == 0), stop=(idx == len(_OUT_CHUNKS) - 1))
        y_sb = work.tile([80, 1], F32, tag="y", name="y_sb")
        nc.vector.tensor_copy(y_sb[:], O[:])
        nc.sync.dma_start(out=ydram[:], in_=y_sb[:])

    nc.compile()
    return nc


_NC_CACHE = None


def _get_nc():
    global _NC_CACHE
    if _NC_CACHE is None:
        _NC_CACHE = _build_nc()
    return _NC_CACHE


def kernel(**inputs) -> np.ndarray:
    from concourse.bass_utils import run_bass_kernel_spmd

    nc = _get_nc()
    in_map = _pack(inputs)
    in_maps = [in_map for _ in range(N_CORES)]
    res = run_bass_kernel_spmd(nc, in_maps, list(range(N_CORES)))
    y = np.asarray(res.results[0]["y"]).reshape(-1)
    return y.reshape(1, 4, 20).astype(np.float32)
